# revision 18
# baseline (speedup 1.0000x reference)
"""Trainium2 Bass kernel for nn_CrossAttention_33913061769426.

Cross-attention with AdaptiveLayerNorm (AlphaFold3-style), B=2 H=4 Q=K=3072 C=128.

Sharding: 8 cores = (batch b, Q-quarter). Each core computes 768 full output
rows (all 4 heads), so the host just concatenates per-core outputs.

Key device-side design:
  - Softmax without max-subtraction: for unmasked rows (mask_q==1) logits are
    O(+-10) so exp() is safe; masked rows (mask_q==0) reduce EXACTLY (in f32,
    because 1e9 + x rounds to 1e9) to a uniform average of v over mask_k==0
    columns, which we substitute via m0 = sum_{mask_k==0} v and a host-built
    U = (1-mask_q)/n0 row.  Softmax division is deferred past the
    weights@v matmul (row scaling commutes), with the row-sums produced for
    free by a ones-column appended to v (M=33 col-strips at partition 0/64).
  - logits matmuls (contraction = Dh = 32) are row-packed 4-heads-concurrent
    via tile_position row strips.
  - exp runs AFTER the PE transpose, directly PSUM->SBUF, so no extra copy
    pass is needed.
"""

from contextlib import ExitStack

import numpy as np

import concourse.bass as bass
import concourse.mybir as mybir
import concourse.tile as tile
from concourse import bacc
from concourse.bass_utils import run_bass_kernel_spmd
from concourse.masks import make_identity

F32 = mybir.dt.float32
AX = mybir.AxisListType
OP = mybir.AluOpType
AF = mybir.ActivationFunctionType

B, Q, K, C, H = 2, 3072, 3072, 128, 4
Dh = C // H
QS = Q // 4          # 768 q-rows per core
HALF = QS // 2       # 384
NKT = K // 128       # 24 k-tiles
KC = 512             # logits k-chunk
NKC = K // KC        # 12
EPS = 1e-5

N_CORES = 8


def build_kernel(phase='full'):
    nc = bacc.Bacc("TRN2", target_bir_lowering=False, debug=False,
                   num_devices=N_CORES)

    def din(name, shape):
        return nc.dram_tensor(name, list(shape), F32, kind="ExternalInput").ap()

    # per-core inputs
    x_q = din("x_q", (QS, C))
    scq = din("scq", (QS, C))
    x_k = din("x_k", (K, C))
    sck = din("sck", (K, C))
    pair = din("pair", (H, QS, K))
    mq_row = din("mq_row", (1, QS))       # f32 mask_q slice
    u_row = din("u_row", (1, QS))         # (1-mq)/n0
    u0_col = din("u0_col", (128, NKT))    # (1-mk) rearranged (t p) -> p t
    # weights (host-prepped)
    Wsq = din("Wsq", (C, C))
    Wbq = din("Wbq", (C, C))
    Wsk = din("Wsk", (C, C))
    Wbk = din("Wbk", (C, C))
    bsq = din("bsq", (C, 1))
    bsk = din("bsk", (C, 1))
    gq = din("gq", (C, 1))                # gamma_cq
    gk = din("gk", (C, 1))                # gamma_ck
    Wqp = din("Wqp", (C, C))              # pre-scaled by Dh^-0.5
    bqp = din("bqp", (C, 1))              # pre-scaled
    Wkp = din("Wkp", (C, C))
    Wvp = din("Wvp", (C, C))
    WgA = din("WgA", (C, C))
    WgB = din("WgB", (C, C))
    Wt2A = din("Wt2A", (C, C))            # rows permuted+zeroed
    Wt2B = din("Wt2B", (C, C))
    Wzc = din("Wzc", (C, C))
    bzc = din("bzc", (C, 1))

    out_d = nc.dram_tensor("out", [QS, C], F32, kind="ExternalOutput").ap()

    with tile.TileContext(nc) as tc, ExitStack() as stack:
        const = stack.enter_context(tc.tile_pool(name="const", bufs=1))

        identity = const.tile([128, 128], F32)
        make_identity(nc, identity)

        def load_const(ap_d, shape):
            t = const.tile(list(shape), F32, tag=f"c_{ap_d.name}",
                           name=f"c_{ap_d.name}")
            nc.gpsimd.dma_start(out=t, in_=ap_d)
            return t

        Wsq_s = load_const(Wsq, (C, C))
        Wbq_s = load_const(Wbq, (C, C))
        Wsk_s = load_const(Wsk, (C, C))
        Wbk_s = load_const(Wbk, (C, C))
        bsq_s = load_const(bsq, (C, 1))
        bsk_s = load_const(bsk, (C, 1))
        gq_s = load_const(gq, (C, 1))
        gk_s = load_const(gk, (C, 1))
        Wq_s = load_const(Wqp, (C, C))
        bq_s = load_const(bqp, (C, 1))
        Wk_s = load_const(Wkp, (C, C))
        Wv_s = load_const(Wvp, (C, C))
        WgA_s = load_const(WgA, (C, C))
        WgB_s = load_const(WgB, (C, C))
        Wt2A_s = load_const(Wt2A, (C, C))
        Wt2B_s = load_const(Wt2B, (C, C))
        Wzc_s = load_const(Wzc, (C, C))
        bzc_s = load_const(bzc, (C, 1))
        u0_s = load_const(u0_col, (128, NKT))

        eps_s = const.tile([128, 1], F32)
        nc.vector.memset(eps_s, EPS)
        ones_s = const.tile([128, 64], F32)
        nc.vector.memset(ones_s, 1.0)

        # mask_q row replicated at partitions 32 and 96 (aligned with the
        # sum-rows of the wa PSUM accumulators)
        mqp_s = const.tile([128, QS], F32)
        nc.gpsimd.dma_start(out=mqp_s[32:33, :], in_=mq_row)
        nc.gpsimd.dma_start(out=mqp_s[96:97, :], in_=mq_row)
        # U broadcast to all partitions
        U_s = const.tile([128, QS], F32)
        nc.gpsimd.dma_start(out=U_s, in_=u_row.to_broadcast((128, QS)))

        # persistent activations
        persist = stack.enter_context(tc.tile_pool(name="persist", bufs=1))
        xqfT = persist.tile([C, QS], F32)     # adaptive-LN'd x_q, transposed
        scqT = persist.tile([C, QS], F32)     # raw single_cond_q, transposed
        qT = persist.tile([C, QS], F32)       # q projection (heads stacked)
        xkfT = persist.tile([C, K], F32)
        kT = persist.tile([C, K], F32)
        vA = persist.tile([128, NKT, 128], F32)   # heads {0,1} + ones cols
        vB = persist.tile([128, NKT, 128], F32)   # heads {2,3} + ones cols
        m0A_s = persist.tile([128, 1], F32)
        m0B_s = persist.tile([128, 1], F32)

        # ---------------- Phase 1: LayerNorm + transposes ----------------
        def ln_tiles(src, n_tiles, pool, psum_pool, dst_T, gamma=None,
                     also_raw_T=None):
            """LayerNorm rows of src ([n*128, C]) then transpose into dst_T
            ([C, n*128]).  If also_raw_T given, also transpose raw input."""
            for t in range(n_tiles):
                raw = pool.tile([128, C], F32, tag="ln_raw")
                nc.sync.dma_start(out=raw, in_=src[t * 128:(t + 1) * 128, :])
                stats = pool.tile([128, 6], F32, tag="ln_stats")
                nc.vector.bn_stats(out=stats, in_=raw)
                mv = pool.tile([128, 2], F32, tag="ln_mv")
                nc.vector.bn_aggr(out=mv, in_=stats)
                std = pool.tile([128, 1], F32, tag="ln_std")
                nc.scalar.activation(out=std, in_=mv[:, 1:2], func=AF.Sqrt,
                                     bias=eps_s)
                rstd = pool.tile([128, 1], F32, tag="ln_rstd")
                nc.vector.reciprocal(out=rstd, in_=std)
                xn = pool.tile([128, C], F32, tag="ln_xn")
                nc.vector.tensor_scalar(out=xn, in0=raw, scalar1=mv[:, 0:1],
                                        scalar2=rstd, op0=OP.subtract,
                                        op1=OP.mult)
                ps = psum_pool.tile([128, 128], F32, tag="ps")
                nc.tensor.transpose(ps, xn, identity)
                if gamma is not None:
                    nc.scalar.mul(out=dst_T[:, t * 128:(t + 1) * 128], in_=ps,
                                  mul=gamma)
                else:
                    nc.scalar.copy(out=dst_T[:, t * 128:(t + 1) * 128], in_=ps)
                if also_raw_T is not None:
                    ps2 = psum_pool.tile([128, 128], F32, tag="ps")
                    nc.tensor.transpose(ps2, raw, identity)
                    nc.scalar.copy(out=also_raw_T[:, t * 128:(t + 1) * 128],
                                   in_=ps2)

        with tc.tile_pool(name="prep", bufs=4) as prep, \
             tc.tile_pool(name="prep_big", bufs=1) as prep_big, \
             tc.tile_pool(name="prep_ps", bufs=4, space="PSUM") as prep_ps, \
             tc.tile_pool(name="m0_ps", bufs=1, space="PSUM") as m0_ps:

            xnqT = prep_big.tile([C, QS], F32, tag="xnqT")
            cnqT = prep_big.tile([C, QS], F32, tag="cnqT")
            ln_tiles(x_q, QS // 128, prep, prep_ps, xnqT)
            ln_tiles(scq, QS // 128, prep, prep_ps, cnqT, gamma=gq_s,
                     also_raw_T=scqT)
            xnkT = prep_big.tile([C, K], F32, tag="xnkT")
            cnkT = prep_big.tile([C, K], F32, tag="cnkT")
            ln_tiles(x_k, NKT, prep, prep_ps, xnkT)
            ln_tiles(sck, NKT, prep, prep_ps, cnkT, gamma=gk_s)

            # ---------- Phase 2: adaptive-LN combine (transposed domain) ----
            def adaptive(xnT, cnT, Ws_s, Wb_s, bs_s, dstT, n):
                for c0 in range(0, n, 512):
                    w = min(512, n - c0)
                    sl = slice(c0, c0 + w)
                    ps = prep_ps.tile([128, 512], F32, tag="ps")
                    nc.tensor.matmul(ps[:, :w], Ws_s, cnT[:, sl], start=True,
                                     stop=True)
                    sig = prep.tile([128, 512], F32, tag="ad_sig")
                    nc.scalar.activation(out=sig[:, :w], in_=ps[:, :w],
                                         func=AF.Sigmoid, bias=bs_s)
                    ps2 = prep_ps.tile([128, 512], F32, tag="ps")
                    nc.tensor.matmul(ps2[:, :w], Wb_s, cnT[:, sl], start=True,
                                     stop=True)
                    tmp = prep.tile([128, 512], F32, tag="ad_tmp")
                    nc.vector.tensor_tensor(out=tmp[:, :w], in0=sig[:, :w],
                                            in1=xnT[:, sl], op=OP.mult)
                    nc.vector.tensor_tensor(out=dstT[:, sl], in0=tmp[:, :w],
                                            in1=ps2[:, :w], op=OP.add)

            adaptive(xnqT, cnqT, Wsq_s, Wbq_s, bsq_s, xqfT, QS)
            adaptive(xnkT, cnkT, Wsk_s, Wbk_s, bsk_s, xkfT, K)

            # ---------- Phase 3: projections ----------
            # qT = Wq'.T @ xqfT + bq'   (heads stacked on partitions)
            for c0 in range(0, QS, 512):
                w = min(512, QS - c0)
                ps = prep_ps.tile([128, 512], F32, tag="ps")
                nc.tensor.matmul(ps[:, :w], Wq_s, xqfT[:, c0:c0 + w],
                                 start=True, stop=True)
                nc.scalar.add(out=qT[:, c0:c0 + w], in_=ps[:, :w], add=bq_s)
            for c0 in range(0, K, 512):
                ps = prep_ps.tile([128, 512], F32, tag="ps")
                nc.tensor.matmul(ps, Wk_s, xkfT[:, c0:c0 + 512], start=True,
                                 stop=True)
                nc.scalar.copy(out=kT[:, c0:c0 + 512], in_=ps)
            # v in [K, heads*Dh] layout, split into the paired strip tensors
            for kt in range(NKT):
                ps = prep_ps.tile([128, 128], F32, tag="ps")
                nc.tensor.matmul(ps, xkfT[:, kt * 128:(kt + 1) * 128], Wv_s,
                                 start=True, stop=True)
                nc.scalar.copy(out=vA[:, kt, 0:32], in_=ps[:, 0:32])
                nc.scalar.copy(out=vA[:, kt, 64:96], in_=ps[:, 32:64])
                nc.scalar.copy(out=vB[:, kt, 0:32], in_=ps[:, 64:96])
                nc.scalar.copy(out=vB[:, kt, 64:96], in_=ps[:, 96:128])
            nc.vector.memset(vA[:, :, 32:33], 1.0)
            nc.vector.memset(vA[:, :, 96:97], 1.0)
            nc.vector.memset(vB[:, :, 32:33], 1.0)
            nc.vector.memset(vB[:, :, 96:97], 1.0)

            # m0 = sum_{mask_k==0} v  (+ n0 in the ones rows, unused)
            ps_m0A = m0_ps.tile([128, 1], F32, tag="m0A")
            ps_m0B = m0_ps.tile([128, 1], F32, tag="m0B")
            nc.vector.memset(ps_m0A, 0.0)
            nc.vector.memset(ps_m0B, 0.0)
            for kt in range(NKT):
                for (pst, vt) in ((ps_m0A, vA), (ps_m0B, vB)):
                    for cb in (0, 64):
                        nc.tensor.matmul(
                            pst[cb:cb + 33, :], vt[:, kt, cb:cb + 33],
                            u0_s[:, kt:kt + 1], start=False,
                            stop=(kt == NKT - 1), tile_position=(0, cb),
                            skip_group_check=True)
            nc.vector.tensor_copy(out=m0A_s, in_=ps_m0A)
            nc.vector.tensor_copy(out=m0B_s, in_=ps_m0B)

        if phase == 'prep':
            with tc.tile_pool(name="dbg", bufs=1) as dbg:
                o1 = dbg.tile([128, C], F32)
                nc.vector.tensor_copy(out=o1, in_=xqfT[:, 0:128])
                nc.sync.dma_start(out=out_d[0:128, :], in_=o1)
                o2 = dbg.tile([128, C], F32)
                nc.vector.tensor_copy(out=o2, in_=vA[:, 0, :])
                nc.sync.dma_start(out=out_d[128:256, :], in_=o2)
            phases_on = False
        else:
            phases_on = True

        # ---------------- Phase 4: attention (per q-half) ----------------
        for hf in range(2 if phases_on else 0):  # noqa: E501
            qh = hf * HALF
            with tc.tile_pool(name=f"wa_ps{hf}", bufs=1, space="PSUM") as wa_ps:
                psum_wa = []
                for p in range(2):  # pair A (h0,h1), pair B (h2,h3)
                    wa_t = wa_ps.tile([128, HALF], F32, tag=f"wa{p}",
                                      name=f"wa{hf}_{p}")
                    psum_wa.append(wa_t)
                for p in range(2):
                    nc.vector.memset(psum_wa[p], 0.0)

                with tc.tile_pool(name=f"att{hf}", bufs=6) as att, \
                     tc.tile_pool(name=f"attE{hf}", bufs=4) as attE, \
                     tc.tile_pool(name=f"pairp{hf}", bufs=6) as pairp, \
                     tc.tile_pool(name=f"psL{hf}", bufs=1, space="PSUM") as psL_pool, \
                     tc.tile_pool(name=f"psT{hf}", bufs=2, space="PSUM") as psT_pool:

                    for kc in range(NKC):
                        S_tiles = []
                        for qt in range(3):
                            q0 = qh + qt * 128
                            pr = pairp.tile([128, H, KC], F32, tag="pair")
                            nc.sync.dma_start(
                                out=pr,
                                in_=pair[:, q0:q0 + 128,
                                         kc * KC:(kc + 1) * KC].rearrange(
                                             "h q k -> q h k"))
                            S = att.tile([128, H, KC], F32, tag="S")
                            for h in range(4):
                                psL = psL_pool.tile([128, KC], F32,
                                                    tag=f"psL{h}",
                                                    name=f"psL{h}")
                                nc.tensor.matmul(
                                    psL,
                                    qT[32 * h:32 * h + 32, q0:q0 + 128],
                                    kT[32 * h:32 * h + 32,
                                       kc * KC:(kc + 1) * KC],
                                    start=True, stop=True,
                                    tile_position=(32 * h, 0))
                                nc.vector.tensor_tensor(out=S[:, h, :],
                                                        in0=psL,
                                                        in1=pr[:, h, :],
                                                        op=OP.add)
                            S_tiles.append(S)
                        for h in range(4):
                            for ktl in range(KC // 128):
                                kt = (KC // 128) * kc + ktl
                                psT = psT_pool.tile([128, HALF], F32,
                                                    tag="psT")
                                for qt in range(3):
                                    nc.tensor.matmul(
                                        psT[:, qt * 128:(qt + 1) * 128],
                                        S_tiles[qt][:, h,
                                                    ktl * 128:(ktl + 1) * 128],
                                        identity, is_transpose=True,
                                        start=(qt == 0), stop=(qt == 2),
                                        skip_group_check=True)
                                E = attE.tile([128, HALF], F32, tag="E")
                                nc.scalar.activation(out=E, in_=psT,
                                                     func=AF.Exp)
                                vt = vA if h < 2 else vB
                                cb = 0 if h % 2 == 0 else 64
                                nc.tensor.matmul(
                                    psum_wa[h // 2][cb:cb + 33, :],
                                    vt[:, kt, cb:cb + 33], E,
                                    start=False, stop=(kt == NKT - 1),
                                    tile_position=(0, cb),
                                    skip_group_check=True)

                if phase != 'full':
                    with tc.tile_pool(name=f"dbgA{hf}", bufs=1) as dbgA:
                        oa = dbgA.tile([128, 128], F32, tag="oa",
                                       name=f"oa{hf}")
                        nc.vector.tensor_copy(out=oa, in_=psum_wa[0][:, 0:128])
                        nc.sync.dma_start(
                            out=out_d[hf * 128:(hf + 1) * 128, :], in_=oa)
                    continue
                # -------- finalize half --------
                with tc.tile_pool(name=f"fin{hf}", bufs=2) as fin, \
                     tc.tile_pool(name=f"fin_ps{hf}", bufs=1,
                                  space="PSUM") as fin_ps:
                    # r1 = mask_q / rowsum, broadcast to the head strips
                    # via PE outer-product (ones[64] x r1row)
                    r1b = []
                    for p in range(2):
                        r1b_t = fin.tile([128, HALF], F32, tag="r1b",
                                         name=f"r1b{hf}_{p}")
                        r1b.append(r1b_t)
                    for p in range(2):
                        rt = fin.tile([128, HALF], F32, tag="rt")
                        for (pp, sl, tp) in ((32, slice(0, 64), (32, 0)),
                                             (96, slice(64, 128), (96, 64))):
                            nc.vector.reciprocal(
                                out=rt[pp:pp + 1, :],
                                in_=psum_wa[p][pp:pp + 1, :])
                            nc.vector.tensor_tensor(
                                out=rt[pp:pp + 1, :], in0=rt[pp:pp + 1, :],
                                in1=mqp_s[pp:pp + 1, qh:qh + HALF], op=OP.mult)
                            ps_r1 = fin_ps.tile([128, HALF], F32,
                                                tag=f"r1ps{pp}",
                                                name=f"r1ps{hf}_{p}_{pp}")
                            nc.tensor.matmul(
                                ps_r1[sl, :], ones_s[pp:pp + 1, :],
                                rt[pp:pp + 1, :], start=True, stop=True,
                                tile_position=tp)
                            nc.scalar.copy(out=r1b[p][sl, :],
                                           in_=ps_r1[sl, :])
                    gated = []
                    for p in range(2):
                        Wg_s = WgA_s if p == 0 else WgB_s
                        m0_s = m0A_s if p == 0 else m0B_s
                        ps_g = fin_ps.tile([128, HALF], F32, tag="ps_g")
                        nc.tensor.matmul(ps_g, Wg_s, xqfT[:, qh:qh + HALF],
                                         start=True, stop=True)
                        g_sb = fin.tile([128, HALF], F32, tag="g_sb")
                        nc.scalar.activation(out=g_sb, in_=ps_g,
                                             func=AF.Sigmoid)
                        gt = fin.tile([128, HALF], F32, tag="gt")
                        nc.vector.tensor_tensor(out=gt, in0=psum_wa[p],
                                                in1=r1b[p], op=OP.mult)
                        nc.vector.scalar_tensor_tensor(
                            out=gt, in0=U_s[:, qh:qh + HALF], scalar=m0_s,
                            in1=gt, op0=OP.mult, op1=OP.add)
                        nc.vector.tensor_tensor(out=gt, in0=gt, in1=g_sb,
                                                op=OP.mult)
                        gated.append(gt)
                    ps_o = fin_ps.tile([128, HALF], F32, tag="ps_o")
                    nc.tensor.matmul(ps_o, Wt2A_s, gated[0], start=True,
                                     stop=False)
                    nc.tensor.matmul(ps_o, Wt2B_s, gated[1], start=False,
                                     stop=True)
                    ps_z = fin_ps.tile([128, HALF], F32, tag="ps_z")
                    nc.tensor.matmul(ps_z, Wzc_s, scqT[:, qh:qh + HALF],
                                     start=True, stop=True)
                    z_sb = fin.tile([128, HALF], F32, tag="z_sb")
                    nc.scalar.activation(out=z_sb, in_=ps_z, func=AF.Sigmoid,
                                         bias=bzc_s)
                    fT = fin.tile([128, HALF], F32, tag="fT")
                    nc.vector.tensor_tensor(out=fT, in0=ps_o, in1=z_sb,
                                            op=OP.mult)
                    for qt in range(3):
                        ps_f = fin_ps.tile([128, 128], F32, tag="ps_f")
                        nc.tensor.matmul(ps_f, fT[:, qt * 128:(qt + 1) * 128],
                                         identity, is_transpose=True,
                                         start=True, stop=True)
                        o_sb = fin.tile([128, 128], F32, tag="o_sb")
                        nc.scalar.copy(out=o_sb, in_=ps_f)
                        nc.sync.dma_start(
                            out=out_d[qh + qt * 128:qh + (qt + 1) * 128, :],
                            in_=o_sb)

    nc.finalize()
    return nc


_NC = None


def _get_nc():
    global _NC
    if _NC is None:
        _NC = build_kernel()
    return _NC


def kernel(x_q, x_k, mask_q, mask_k, pair_logits, single_cond_q, single_cond_k,
           gamma_cq, Wsq, bsq, Wbq, gamma_ck, Wsk, bsk, Wbk,
           Wq, bq, Wk, Wv, Wg, Wt2, Wzc, bzc):
    x_q = np.asarray(x_q, np.float32)
    x_k = np.asarray(x_k, np.float32)
    pair_logits = np.asarray(pair_logits, np.float32)
    single_cond_q = np.asarray(single_cond_q, np.float32)
    single_cond_k = np.asarray(single_cond_k, np.float32)
    mask_q = np.asarray(mask_q)
    mask_k = np.asarray(mask_k)

    scl = np.float32(Dh ** -0.5)
    Wq_f = (np.asarray(Wq, np.float32).reshape(C, C) * scl)
    bq_f = (np.asarray(bq, np.float32).reshape(C, 1) * scl)
    Wk_f = np.asarray(Wk, np.float32).reshape(C, C)
    Wv_f = np.asarray(Wv, np.float32).reshape(C, C)
    Wg_f = np.asarray(Wg, np.float32)
    Wt2_f = np.asarray(Wt2, np.float32)

    # head-pair permuted gating / output-projection weights
    WgA = np.zeros((C, C), np.float32)
    WgB = np.zeros((C, C), np.float32)
    WgA[:, 0:32] = Wg_f[:, 0:32]
    WgA[:, 64:96] = Wg_f[:, 32:64]
    WgB[:, 0:32] = Wg_f[:, 64:96]
    WgB[:, 64:96] = Wg_f[:, 96:128]
    Wt2A = np.zeros((C, C), np.float32)
    Wt2B = np.zeros((C, C), np.float32)
    Wt2A[0:32, :] = Wt2_f[0:32, :]
    Wt2A[64:96, :] = Wt2_f[32:64, :]
    Wt2B[0:32, :] = Wt2_f[64:96, :]
    Wt2B[64:96, :] = Wt2_f[96:128, :]

    common = {
        "Wsq": np.asarray(Wsq, np.float32),
        "Wbq": np.asarray(Wbq, np.float32),
        "Wsk": np.asarray(Wsk, np.float32),
        "Wbk": np.asarray(Wbk, np.float32),
        "bsq": np.asarray(bsq, np.float32).reshape(C, 1),
        "bsk": np.asarray(bsk, np.float32).reshape(C, 1),
        "gq": np.asarray(gamma_cq, np.float32).reshape(C, 1),
        "gk": np.asarray(gamma_ck, np.float32).reshape(C, 1),
        "Wqp": Wq_f, "bqp": bq_f, "Wkp": Wk_f, "Wvp": Wv_f,
        "WgA": WgA, "WgB": WgB, "Wt2A": Wt2A, "Wt2B": Wt2B,
        "Wzc": np.asarray(Wzc, np.float32),
        "bzc": np.asarray(bzc, np.float32).reshape(C, 1),
    }

    in_maps = []
    for core in range(N_CORES):
        b = core // 4
        quarter = core % 4
        q0 = quarter * QS
        mq = mask_q[b].astype(np.float32)
        mk = mask_k[b].astype(np.float32)
        n0 = float((1.0 - mk).sum())
        if n0 > 0:
            u = ((1.0 - mq[q0:q0 + QS]) / np.float32(n0)).astype(np.float32)
            mq_eff = mq[q0:q0 + QS]
        else:
            u = np.zeros(QS, np.float32)
            mq_eff = np.ones(QS, np.float32)
        in_maps.append({
            "x_q": np.ascontiguousarray(x_q[b, q0:q0 + QS]),
            "scq": np.ascontiguousarray(single_cond_q[b, q0:q0 + QS]),
            "x_k": np.ascontiguousarray(x_k[b]),
            "sck": np.ascontiguousarray(single_cond_k[b]),
            "pair": np.ascontiguousarray(pair_logits[b, :, q0:q0 + QS, :]),
            "mq_row": mq_eff.reshape(1, QS).copy(),
            "u_row": u.reshape(1, QS),
            "u0_col": np.ascontiguousarray(
                (1.0 - mk).astype(np.float32).reshape(NKT, 128).T),
            **common,
        })

    nc = _get_nc()
    global _last_in_maps
    _last_in_maps = in_maps
    res = run_bass_kernel_spmd(nc, in_maps, core_ids=list(range(N_CORES)))
    out = np.zeros((B, Q, C), np.float32)
    for core in range(N_CORES):
        b = core // 4
        q0 = (core % 4) * QS
        out[b, q0:q0 + QS] = res.results[core]["out"]
    return out


# revision 25
# speedup vs baseline: 33.9372x; 33.9372x over previous
"""Trainium2 Bass kernel for nn_CrossAttention_33913061769426.

Cross-attention with AdaptiveLayerNorm (AlphaFold3-style), B=2 H=4 Q=K=3072 C=128.

Sharding: 8 cores = (batch b, Q-quarter). Each core computes 768 full output
rows (all 4 heads), so the host just concatenates per-core outputs.

Device-side design notes:
  - Softmax without max-subtraction: for unmasked rows (mask_q==1) logits are
    O(+-10) so exp() is safe; masked rows (mask_q==0) reduce EXACTLY (in f32,
    because 1e9 + x rounds to 1e9) to a uniform average of v over mask_k==0
    columns, substituted via m0 = sum_{mask_k==0} v and a host-built
    U = (1-mask_q)/n0 row.  Softmax division is deferred past the weights@v
    matmul (row scaling commutes), with row-sums produced for free by a ones
    column appended to v (M=33 col-strips at partitions 0/64 — col strips may
    share a PSUM bank; row strips may NOT).
  - logits matmuls (contraction Dh=32) are row-packed 4-heads-concurrent via
    tile_position row strips, each head into its OWN PSUM bank (concurrent
    row-strip matmuls sharing a bank wedge the device).
  - exp runs after the PE transpose, directly PSUM->SBUF (no copy pass).
  - hot matmuls run as float32r (TF32-ish): 1 cycle/row vs 4 for fp32.
"""

from contextlib import ExitStack

import numpy as np

import concourse.bass as bass  # noqa: F401
import concourse.mybir as mybir
import concourse.tile as tile
from concourse import bacc
from concourse.bass_utils import run_bass_kernel_spmd
from concourse.masks import make_identity

F32 = mybir.dt.float32
F32R = mybir.dt.float32r
OP = mybir.AluOpType
AF = mybir.ActivationFunctionType

B, Q, K, C, H = 2, 3072, 3072, 128, 4
Dh = C // H
QS = Q // 4          # 768 q-rows per core
HALF = QS // 2       # 384
NKT = K // 128       # 24 k-tiles
KC = 512             # logits k-chunk
NKC = K // KC        # 6
EPS = 1e-5

N_CORES = 8


def build_kernel():
    nc = bacc.Bacc("TRN2", target_bir_lowering=False, debug=False,
                   num_devices=N_CORES)

    def f32view(ap):
        return ap.bitcast(F32)

    def din(name, shape, dt=F32):
        return nc.dram_tensor(name, list(shape), dt, kind="ExternalInput").ap()

    # per-core inputs
    x_q = din("x_q", (QS, C))
    scq = din("scq", (QS, C))
    x_k = din("x_k", (K, C))
    sck = din("sck", (K, C))
    pair = din("pair", (H, QS, K))
    mq_row = din("mq_row", (1, QS))       # f32 mask_q slice
    u_row = din("u_row", (1, QS))         # (1-mq)/n0
    u0_col = din("u0_col", (128, NKT))    # (1-mk) as (t p) -> p t
    # weights (host-prepped).  Matmul stationaries are f32r.
    Wsq = din("Wsq", (C, C), F32R)
    Wbq = din("Wbq", (C, C), F32R)
    Wsk = din("Wsk", (C, C), F32R)
    Wbk = din("Wbk", (C, C), F32R)
    bsq = din("bsq", (C, 1))
    bsk = din("bsk", (C, 1))
    gq = din("gq", (C, 1))                # gamma_cq
    gk = din("gk", (C, 1))                # gamma_ck
    Wqp = din("Wqp", (C, C), F32R)        # pre-scaled by Dh^-0.5
    bqp = din("bqp", (C, 1))              # pre-scaled
    Wkp = din("Wkp", (C, C), F32R)
    Wvp = din("Wvp", (C, C), F32R)
    WgA = din("WgA", (C, C), F32R)
    WgB = din("WgB", (C, C), F32R)
    Wt2A = din("Wt2A", (C, C), F32R)      # rows permuted + zeroed
    Wt2B = din("Wt2B", (C, C), F32R)
    Wzc = din("Wzc", (C, C), F32R)
    bzc = din("bzc", (C, 1))

    out_d = nc.dram_tensor("out", [QS, C], F32, kind="ExternalOutput").ap()

    with tile.TileContext(nc) as tc, ExitStack() as stack:
        const = stack.enter_context(tc.tile_pool(name="const", bufs=1))

        identity = const.tile([128, 128], F32)
        make_identity(nc, identity)
        ident_r = const.tile([128, 128], F32R)
        nc.scalar.copy(out=ident_r, in_=identity)

        def load_const(ap_d, shape, dt=F32R):
            t = const.tile(list(shape), dt, tag=f"c_{ap_d.name}",
                           name=f"c_{ap_d.name}")
            nc.gpsimd.dma_start(out=t, in_=ap_d)
            return t

        Wsq_s = load_const(Wsq, (C, C))
        Wbq_s = load_const(Wbq, (C, C))
        Wsk_s = load_const(Wsk, (C, C))
        Wbk_s = load_const(Wbk, (C, C))
        bsq_s = load_const(bsq, (C, 1), F32)
        bsk_s = load_const(bsk, (C, 1), F32)
        gq_s = load_const(gq, (C, 1), F32)
        gk_s = load_const(gk, (C, 1), F32)
        Wq_s = load_const(Wqp, (C, C))
        bq_s = load_const(bqp, (C, 1), F32)
        Wk_s = load_const(Wkp, (C, C))
        Wv_s = load_const(Wvp, (C, C))
        WgA_s = load_const(WgA, (C, C))
        WgB_s = load_const(WgB, (C, C))
        Wt2A_s = load_const(Wt2A, (C, C))
        Wt2B_s = load_const(Wt2B, (C, C))
        Wzc_s = load_const(Wzc, (C, C))
        bzc_s = load_const(bzc, (C, 1), F32)
        u0_s = load_const(u0_col, (128, NKT), F32)

        eps_s = const.tile([128, 1], F32)
        nc.vector.memset(eps_s, EPS)
        ones_s = const.tile([128, 64], F32)
        nc.vector.memset(ones_s, 1.0)

        # mask_q row replicated at partitions 32 and 96 (the sum rows of the
        # wa PSUM accumulators); U broadcast to all partitions
        mqp_s = const.tile([128, QS], F32)
        nc.gpsimd.dma_start(out=mqp_s[32:33, :], in_=mq_row)
        nc.gpsimd.dma_start(out=mqp_s[96:97, :], in_=mq_row)
        U_s = const.tile([128, QS], F32)
        nc.gpsimd.dma_start(out=U_s, in_=u_row.to_broadcast((128, QS)))

        # persistent activations (f32r: consumed by f32r matmuls)
        persist = stack.enter_context(tc.tile_pool(name="persist", bufs=1))
        xqfT = persist.tile([C, QS], F32R)    # adaptive-LN'd x_q, transposed
        scqT = persist.tile([C, QS], F32R)    # raw single_cond_q, transposed
        qT = persist.tile([C, QS], F32R)      # q projection (heads stacked)
        xkfT = persist.tile([C, K], F32R)
        kT = persist.tile([C, K], F32R)
        # per-head v, zero-padded so every wa matmul writes from partition 0
        # (fp32r matmuls require dst base partition 0):
        #   even heads: cols [0:32]=v, [32:34]=1, rest 0       (M=34)
        #   odd heads:  cols [0:64]=0, [64:96]=v, [96:98]=1    (M=98)
        vh = []
        for h in range(4):
            vh_t = persist.tile([128, NKT, 128], F32R, tag=f"vh{h}",
                                name=f"vh{h}")
            vh.append(vh_t)
        m0A_s = persist.tile([128, 1], F32)
        m0B_s = persist.tile([128, 1], F32)

        # ---------------- Phase 1: LayerNorm + transposes ----------------
        def ln_tiles(src, n_tiles, pool, psum_pool, dst_T, gamma=None,
                     also_raw_T=None):
            for t in range(n_tiles):
                raw = pool.tile([128, C], F32, tag="ln_raw")
                nc.sync.dma_start(out=raw, in_=src[t * 128:(t + 1) * 128, :])
                stats = pool.tile([128, 6], F32, tag="ln_stats")
                nc.vector.bn_stats(out=stats, in_=raw)
                mv = pool.tile([128, 2], F32, tag="ln_mv")
                nc.vector.bn_aggr(out=mv, in_=stats)
                std = pool.tile([128, 1], F32, tag="ln_std")
                nc.scalar.activation(out=std, in_=mv[:, 1:2], func=AF.Sqrt,
                                     bias=eps_s)
                rstd = pool.tile([128, 1], F32, tag="ln_rstd")
                nc.vector.reciprocal(out=rstd, in_=std)
                xn = pool.tile([128, C], F32, tag="ln_xn")
                nc.vector.tensor_scalar(out=xn, in0=raw, scalar1=mv[:, 0:1],
                                        scalar2=rstd, op0=OP.subtract,
                                        op1=OP.mult)
                ps = psum_pool.tile([128, 128], F32, tag="ps")
                nc.tensor.transpose(ps, xn, identity)
                sl = slice(t * 128, (t + 1) * 128)
                if gamma is not None:
                    nc.scalar.mul(out=dst_T[:, sl], in_=ps, mul=gamma)
                else:
                    nc.scalar.copy(out=dst_T[:, sl], in_=ps)
                if also_raw_T is not None:
                    ps2 = psum_pool.tile([128, 128], F32, tag="ps")
                    nc.tensor.transpose(ps2, raw, identity)
                    nc.scalar.copy(out=also_raw_T[:, sl], in_=ps2)

        with tc.tile_pool(name="prep", bufs=4) as prep, \
             tc.tile_pool(name="prep_big", bufs=1) as prep_big, \
             tc.tile_pool(name="prep_ps", bufs=4, space="PSUM") as prep_ps, \
             tc.tile_pool(name="m0_ps", bufs=1, space="PSUM") as m0_ps:

            xnqT = prep_big.tile([C, QS], F32, tag="xnqT")
            cnqT = prep_big.tile([C, QS], F32R, tag="cnqT")
            ln_tiles(x_q, QS // 128, prep, prep_ps, xnqT)
            ln_tiles(scq, QS // 128, prep, prep_ps, cnqT, gamma=gq_s,
                     also_raw_T=scqT)
            xnkT = prep_big.tile([C, K], F32, tag="xnkT")
            cnkT = prep_big.tile([C, K], F32R, tag="cnkT")
            ln_tiles(x_k, NKT, prep, prep_ps, xnkT)
            ln_tiles(sck, NKT, prep, prep_ps, cnkT, gamma=gk_s)

            # ---------- Phase 2: adaptive-LN combine (transposed domain) ---
            def adaptive(xnT, cnT, Ws_s, Wb_s, bs_s, dstT, n):
                for c0 in range(0, n, 512):
                    w = min(512, n - c0)
                    sl = slice(c0, c0 + w)
                    ps = prep_ps.tile([128, 512], F32, tag="ps")
                    nc.tensor.matmul(ps[:, :w], Ws_s, cnT[:, sl], start=True,
                                     stop=True)
                    sig = prep.tile([128, 512], F32, tag="ad_sig")
                    nc.scalar.activation(out=sig[:, :w], in_=ps[:, :w],
                                         func=AF.Sigmoid, bias=bs_s)
                    ps2 = prep_ps.tile([128, 512], F32, tag="ps")
                    nc.tensor.matmul(ps2[:, :w], Wb_s, cnT[:, sl], start=True,
                                     stop=True)
                    tmp = prep.tile([128, 512], F32, tag="ad_tmp")
                    nc.vector.tensor_tensor(out=tmp[:, :w], in0=sig[:, :w],
                                            in1=xnT[:, sl], op=OP.mult)
                    nc.vector.tensor_tensor(out=dstT[:, sl], in0=tmp[:, :w],
                                            in1=ps2[:, :w], op=OP.add)

            adaptive(xnqT, cnqT, Wsq_s, Wbq_s, bsq_s, xqfT, QS)
            adaptive(xnkT, cnkT, Wsk_s, Wbk_s, bsk_s, xkfT, K)

            # ---------- Phase 3: projections ----------
            for c0 in range(0, QS, 512):
                w = min(512, QS - c0)
                ps = prep_ps.tile([128, 512], F32, tag="ps")
                nc.tensor.matmul(ps[:, :w], Wq_s, xqfT[:, c0:c0 + w],
                                 start=True, stop=True)
                nc.scalar.add(out=qT[:, c0:c0 + w], in_=ps[:, :w], add=bq_s)
            for c0 in range(0, K, 512):
                ps = prep_ps.tile([128, 512], F32, tag="ps")
                nc.tensor.matmul(ps, Wk_s, xkfT[:, c0:c0 + 512], start=True,
                                 stop=True)
                nc.vector.tensor_copy(out=kT[:, c0:c0 + 512], in_=ps)
            # v in [K, heads*Dh] layout, split into per-head padded tensors
            for h in range(4):
                nc.vector.memset(f32view(vh[h]), 0.0)
            nc.vector.memset(f32view(vh[0][:, :, 32:34]), 1.0)
            nc.vector.memset(f32view(vh[1][:, :, 96:98]), 1.0)
            nc.vector.memset(f32view(vh[2][:, :, 32:34]), 1.0)
            nc.vector.memset(f32view(vh[3][:, :, 96:98]), 1.0)
            for kt in range(NKT):
                ps = prep_ps.tile([128, 128], F32, tag="ps")
                nc.tensor.matmul(ps, xkfT[:, kt * 128:(kt + 1) * 128], Wv_s,
                                 start=True, stop=True)
                nc.vector.tensor_copy(out=vh[0][:, kt, 0:32], in_=ps[:, 0:32])
                nc.vector.tensor_copy(out=vh[1][:, kt, 64:96],
                                      in_=ps[:, 32:64])
                nc.vector.tensor_copy(out=vh[2][:, kt, 0:32],
                                      in_=ps[:, 64:96])
                nc.vector.tensor_copy(out=vh[3][:, kt, 64:96],
                                      in_=ps[:, 96:128])

            # m0 = sum_{mask_k==0} v  (n0 lands in the ones rows, unused)
            ps_m0A = m0_ps.tile([128, 1], F32, tag="m0A")
            ps_m0B = m0_ps.tile([128, 1], F32, tag="m0B")
            nc.vector.memset(ps_m0A, 0.0)
            nc.vector.memset(ps_m0B, 0.0)
            for kt in range(NKT):
                for h in range(4):
                    pst = ps_m0A if h < 2 else ps_m0B
                    m = 34 if h % 2 == 0 else 98
                    nc.tensor.matmul(
                        pst[0:m, :], f32view(vh[h][:, kt, 0:m]),
                        u0_s[:, kt:kt + 1], start=False,
                        stop=(kt == NKT - 1), tile_position=(0, 0),
                        skip_group_check=True)
            nc.vector.tensor_copy(out=m0A_s, in_=ps_m0A)
            nc.vector.tensor_copy(out=m0B_s, in_=ps_m0B)

        # ---------------- Phase 4: attention (per q-half) ----------------
        for hf in range(2):
            qh = hf * HALF
            with tc.tile_pool(name=f"wa_ps{hf}", bufs=1, space="PSUM") as wa_ps:
                psum_wa = []
                for p in range(2):  # pair A (h0,h1), pair B (h2,h3)
                    wa_t = wa_ps.tile([128, HALF], F32, tag=f"wa{p}",
                                      name=f"wa{hf}_{p}")
                    psum_wa.append(wa_t)
                    nc.vector.memset(wa_t, 0.0)

                with tc.tile_pool(name=f"att{hf}", bufs=6) as att, \
                     tc.tile_pool(name=f"attE{hf}", bufs=4) as attE, \
                     tc.tile_pool(name=f"pairp{hf}", bufs=6) as pairp, \
                     tc.tile_pool(name=f"psL{hf}", bufs=2,
                                  space="PSUM") as psL_pool, \
                     tc.tile_pool(name=f"psT{hf}", bufs=2, space="PSUM") as psT_pool:

                    for kc in range(NKC):
                        S_tiles = []
                        for qt in range(3):
                            q0 = qh + qt * 128
                            pr = pairp.tile([128, H, KC], F32, tag="pair")
                            nc.sync.dma_start(
                                out=pr,
                                in_=pair[:, q0:q0 + 128,
                                         kc * KC:(kc + 1) * KC].rearrange(
                                             "h q k -> q h k"))
                            S = att.tile([128, H, KC], F32R, tag="S")
                            for h in range(4):
                                psL = psL_pool.tile([128, KC], F32,
                                                    tag=f"psL{h % 2}",
                                                    name=f"psL{h}")
                                nc.tensor.matmul(
                                    psL,
                                    qT[32 * h:32 * h + 32, q0:q0 + 128],
                                    kT[32 * h:32 * h + 32,
                                       kc * KC:(kc + 1) * KC],
                                    start=True, stop=True,
                                    tile_position=(32 * h, 0))
                                nc.vector.tensor_tensor(out=S[:, h, :],
                                                        in0=psL,
                                                        in1=pr[:, h, :],
                                                        op=OP.add)
                            S_tiles.append(S)
                        for h in range(4):
                            for ktl in range(KC // 128):
                                kt = (KC // 128) * kc + ktl
                                psT = psT_pool.tile([128, HALF], F32R,
                                                    tag="psT")
                                for qt in range(3):
                                    nc.tensor.matmul(
                                        psT[:, qt * 128:(qt + 1) * 128],
                                        S_tiles[qt][:, h,
                                                    ktl * 128:(ktl + 1) * 128],
                                        ident_r, is_transpose=True,
                                        start=(qt == 0), stop=(qt == 2),
                                        skip_group_check=True)
                                E = attE.tile([128, HALF], F32R, tag="E")
                                nc.scalar.activation(out=E,
                                                     in_=f32view(psT),
                                                     func=AF.Exp)
                                m = 34 if h % 2 == 0 else 98
                                nc.tensor.matmul(
                                    psum_wa[h // 2][0:m, :],
                                    vh[h][:, kt, 0:m], E,
                                    start=False, stop=(kt == NKT - 1),
                                    tile_position=(0, 0),
                                    skip_group_check=True)

                # -------- finalize half --------
                with tc.tile_pool(name=f"fin{hf}", bufs=2) as fin, \
                     tc.tile_pool(name=f"fin_ps{hf}", bufs=1,
                                  space="PSUM") as fin_ps:
                    # r1 = mask_q / rowsum, broadcast to the head strips via
                    # PE outer product (ones x r1row)
                    r1b = []
                    for p in range(2):
                        r1b_t = fin.tile([128, HALF], F32, tag="r1b",
                                         name=f"r1b{hf}_{p}")
                        r1b.append(r1b_t)
                    for p in range(2):
                        rt = fin.tile([128, HALF], F32, tag="rt")
                        for (pp, sl, tp) in ((32, slice(0, 64), (32, 0)),
                                             (96, slice(64, 128), (96, 64))):
                            nc.vector.reciprocal(
                                out=rt[pp:pp + 1, :],
                                in_=psum_wa[p][pp:pp + 1, :])
                            nc.vector.tensor_tensor(
                                out=rt[pp:pp + 1, :], in0=rt[pp:pp + 1, :],
                                in1=mqp_s[pp:pp + 1, qh:qh + HALF], op=OP.mult)
                            ps_r1 = fin_ps.tile([128, HALF], F32,
                                                tag=f"r1ps{pp}",
                                                name=f"r1ps{hf}_{p}_{pp}")
                            nc.tensor.matmul(
                                ps_r1[sl, :], ones_s[pp:pp + 1, :],
                                rt[pp:pp + 1, :], start=True, stop=True,
                                tile_position=tp)
                            nc.scalar.copy(out=r1b[p][sl, :],
                                           in_=ps_r1[sl, :])
                    gated = []
                    for p in range(2):
                        Wg_s = WgA_s if p == 0 else WgB_s
                        m0_s = m0A_s if p == 0 else m0B_s
                        ps_g = fin_ps.tile([128, HALF], F32, tag="ps_g")
                        nc.tensor.matmul(ps_g, Wg_s, xqfT[:, qh:qh + HALF],
                                         start=True, stop=True)
                        g_sb = fin.tile([128, HALF], F32, tag="g_sb")
                        nc.scalar.activation(out=g_sb, in_=ps_g,
                                             func=AF.Sigmoid)
                        gt = fin.tile([128, HALF], F32R, tag="gt")
                        nc.vector.tensor_tensor(out=gt, in0=psum_wa[p],
                                                in1=r1b[p], op=OP.mult)
                        nc.vector.scalar_tensor_tensor(
                            out=gt, in0=U_s[:, qh:qh + HALF], scalar=m0_s,
                            in1=f32view(gt), op0=OP.mult, op1=OP.add)
                        nc.vector.tensor_tensor(out=gt, in0=f32view(gt),
                                                in1=g_sb, op=OP.mult)
                        gated.append(gt)
                    ps_o = fin_ps.tile([128, HALF], F32, tag="ps_o")
                    nc.tensor.matmul(ps_o, Wt2A_s, gated[0], start=True,
                                     stop=False)
                    nc.tensor.matmul(ps_o, Wt2B_s, gated[1], start=False,
                                     stop=True)
                    ps_z = fin_ps.tile([128, HALF], F32, tag="ps_z")
                    nc.tensor.matmul(ps_z, Wzc_s, scqT[:, qh:qh + HALF],
                                     start=True, stop=True)
                    z_sb = fin.tile([128, HALF], F32, tag="z_sb")
                    nc.scalar.activation(out=z_sb, in_=ps_z, func=AF.Sigmoid,
                                         bias=bzc_s)
                    fT = fin.tile([128, HALF], F32, tag="fT")
                    nc.vector.tensor_tensor(out=fT, in0=ps_o, in1=z_sb,
                                            op=OP.mult)
                    for qt in range(3):
                        ps_f = fin_ps.tile([128, 128], F32, tag="ps_f")
                        nc.tensor.matmul(ps_f, fT[:, qt * 128:(qt + 1) * 128],
                                         identity, is_transpose=True,
                                         start=True, stop=True)
                        o_sb = fin.tile([128, 128], F32, tag="o_sb")
                        nc.scalar.copy(out=o_sb, in_=ps_f)
                        nc.sync.dma_start(
                            out=out_d[qh + qt * 128:qh + (qt + 1) * 128, :],
                            in_=o_sb)

    nc.finalize()
    return nc


_NC = None
_last_in_maps = None


def _get_nc():
    global _NC
    if _NC is None:
        _NC = build_kernel()
    return _NC


def kernel(x_q, x_k, mask_q, mask_k, pair_logits, single_cond_q, single_cond_k,
           gamma_cq, Wsq, bsq, Wbq, gamma_ck, Wsk, bsk, Wbk,
           Wq, bq, Wk, Wv, Wg, Wt2, Wzc, bzc):
    x_q = np.asarray(x_q, np.float32)
    x_k = np.asarray(x_k, np.float32)
    pair_logits = np.asarray(pair_logits, np.float32)
    single_cond_q = np.asarray(single_cond_q, np.float32)
    single_cond_k = np.asarray(single_cond_k, np.float32)
    mask_q = np.asarray(mask_q)
    mask_k = np.asarray(mask_k)

    scl = np.float32(Dh ** -0.5)
    Wq_f = (np.asarray(Wq, np.float32).reshape(C, C) * scl)
    bq_f = (np.asarray(bq, np.float32).reshape(C, 1) * scl)
    Wk_f = np.asarray(Wk, np.float32).reshape(C, C)
    Wv_f = np.asarray(Wv, np.float32).reshape(C, C)
    Wg_f = np.asarray(Wg, np.float32)
    Wt2_f = np.asarray(Wt2, np.float32)

    # head-pair permuted gating / output-projection weights
    WgA_h = np.zeros((C, C), np.float32)
    WgB_h = np.zeros((C, C), np.float32)
    WgA_h[:, 0:32] = Wg_f[:, 0:32]
    WgA_h[:, 64:96] = Wg_f[:, 32:64]
    WgB_h[:, 0:32] = Wg_f[:, 64:96]
    WgB_h[:, 64:96] = Wg_f[:, 96:128]
    Wt2A_h = np.zeros((C, C), np.float32)
    Wt2B_h = np.zeros((C, C), np.float32)
    Wt2A_h[0:32, :] = Wt2_f[0:32, :]
    Wt2A_h[64:96, :] = Wt2_f[32:64, :]
    Wt2B_h[0:32, :] = Wt2_f[64:96, :]
    Wt2B_h[64:96, :] = Wt2_f[96:128, :]

    common = {
        "Wsq": np.asarray(Wsq, np.float32),
        "Wbq": np.asarray(Wbq, np.float32),
        "Wsk": np.asarray(Wsk, np.float32),
        "Wbk": np.asarray(Wbk, np.float32),
        "bsq": np.asarray(bsq, np.float32).reshape(C, 1),
        "bsk": np.asarray(bsk, np.float32).reshape(C, 1),
        "gq": np.asarray(gamma_cq, np.float32).reshape(C, 1),
        "gk": np.asarray(gamma_ck, np.float32).reshape(C, 1),
        "Wqp": Wq_f, "bqp": bq_f, "Wkp": Wk_f, "Wvp": Wv_f,
        "WgA": WgA_h, "WgB": WgB_h, "Wt2A": Wt2A_h, "Wt2B": Wt2B_h,
        "Wzc": np.asarray(Wzc, np.float32),
        "bzc": np.asarray(bzc, np.float32).reshape(C, 1),
    }

    in_maps = []
    for core in range(N_CORES):
        b = core // 4
        q0 = (core % 4) * QS
        mq = mask_q[b].astype(np.float32)
        mk = mask_k[b].astype(np.float32)
        n0 = float((1.0 - mk).sum())
        if n0 > 0:
            u = ((1.0 - mq[q0:q0 + QS]) / np.float32(n0)).astype(np.float32)
            mq_eff = mq[q0:q0 + QS]
        else:
            u = np.zeros(QS, np.float32)
            mq_eff = np.ones(QS, np.float32)
        in_maps.append({
            "x_q": np.ascontiguousarray(x_q[b, q0:q0 + QS]),
            "scq": np.ascontiguousarray(single_cond_q[b, q0:q0 + QS]),
            "x_k": np.ascontiguousarray(x_k[b]),
            "sck": np.ascontiguousarray(single_cond_k[b]),
            "pair": np.ascontiguousarray(pair_logits[b, :, q0:q0 + QS, :]),
            "mq_row": mq_eff.reshape(1, QS).copy(),
            "u_row": u.reshape(1, QS),
            "u0_col": np.ascontiguousarray(
                (1.0 - mk).astype(np.float32).reshape(NKT, 128).T),
            **common,
        })

    nc = _get_nc()
    global _last_in_maps
    _last_in_maps = in_maps
    res = run_bass_kernel_spmd(nc, in_maps, core_ids=list(range(N_CORES)))
    out = np.zeros((B, Q, C), np.float32)
    for core in range(N_CORES):
        b = core // 4
        q0 = (core % 4) * QS
        out[b, q0:q0 + QS] = res.results[core]["out"]
    return out
